# revision 1
# baseline (speedup 1.0000x reference)
"""AxialShift block on 8 TRN2 NeuronCores (Bass/Tile, SPMD).

Computation (see the nn.Module reference):
    h   = gelu(groupnorm1(conv1x1(x, w1, b1), g1, bt1))
    x_a = axial_shift(pad(h), axis=a) for a in D,H,W  (3 channel chunks
          shifted by -1/0/+1 along the axis, zero boundary)
    y   = sum_a gelu(conv1x1(x_a, w2a, b2a))
    out = conv1x1(groupnorm1(y, g2, bt2), w3, b3)

Sharding: core k -> (b = k//4, d-slices [8k%32, +8)). Halo of 1 D-slice is
recomputed locally (host pre-pads x with zeros at sample edges). GroupNorm
stats are all-reduced across the 4 cores of each sample as 2 scalars.

Per core, h lives in SBUF in a zero-padded layout with one shared zero
row/col between 32x32 planes (stride 33), so the three axial shifts become
plain AP offset reads (W: +-1, H: +-33, D: +-1089). Norm affines are folded
into activation scale/bias (gn1) and host-folded weights + per-channel
epilogue (gn2), so the final conv's matmuls don't wait on the stats
collective. y is spilled to DRAM as bf16 and re-read for the final conv.
"""

import numpy as np

DIM = 384
R = 32
B = 2
EPS = 1e-5

DSH = 8                 # own D-slices per core
DTOT = DSH + 2          # + halo
SLICE = 33 * 33         # padded 32x32 plane with shared zero row/col
HBUF = DTOT * SLICE + 1  # +1 head zero element
TOK_IN = DTOT * R * R   # 10240
NT_IN = TOK_IN // 512   # 20
TOK_OUT = DSH * R * R   # 8192
NT_OUT = TOK_OUT // 512  # 16
NTOT = float(DIM * R * R * R)  # elements per sample for groupnorm

# rows of the packed per-channel vector input
VB1, VG1, VBT1, VB21, VB22, VB23, VAV, VBV = range(8)

_compiled = None


def _build(gelu_func=None, debug=False):
    import concourse.bass as bass
    import concourse.bacc as bacc
    import concourse.tile as tile
    from concourse import mybir

    f32 = mybir.dt.float32
    f32r = mybir.dt.float32r
    bf16 = mybir.dt.bfloat16
    AF = mybir.ActivationFunctionType
    OP = mybir.AluOpType
    GELU = gelu_func if gelu_func is not None else AF.Gelu

    nc = bacc.Bacc("TRN2", target_bir_lowering=False, debug=False, num_devices=8)

    xs = nc.dram_tensor("xs", [DIM, TOK_IN], bf16, kind="ExternalInput")
    w1t = nc.dram_tensor("w1t", [DIM, DIM], bf16, kind="ExternalInput")
    w2lt = nc.dram_tensor("w2lt", [DIM, DIM], bf16, kind="ExternalInput")
    w2tt = nc.dram_tensor("w2tt", [DIM, DIM], bf16, kind="ExternalInput")
    w2ht = nc.dram_tensor("w2ht", [DIM, DIM], bf16, kind="ExternalInput")
    w3t = nc.dram_tensor("w3t", [DIM, DIM], bf16, kind="ExternalInput")
    vecs = nc.dram_tensor("vecs", [8, DIM], f32, kind="ExternalInput")
    hm = nc.dram_tensor("hm", [2], f32, kind="ExternalInput")
    zpad = nc.dram_tensor("zpad", [330], bf16, kind="ExternalInput")
    out_d = nc.dram_tensor("out", [DIM, TOK_OUT], f32, kind="ExternalOutput")
    dbg_h = dbg_y = dbg_s = None
    if debug:
        dbg_h = [nc.dram_tensor(f"dbg_h{m}", [128, HBUF], bf16, kind="ExternalOutput")
                 for m in range(3)]
        dbg_y = [nc.dram_tensor(f"dbg_y{m}", [128, TOK_OUT], bf16, kind="ExternalOutput")
                 for m in range(3)]
        dbg_s = nc.dram_tensor("dbg_s", [128, 10], f32, kind="ExternalOutput")

    y_d = [nc.dram_tensor(f"y_spill{m}", [128, TOK_OUT], bf16) for m in range(3)]
    cc1_in = nc.dram_tensor("cc1_in", [2], f32)
    cc1_out = nc.dram_tensor("cc1_out", [2], f32)
    cc2_in = nc.dram_tensor("cc2_in", [2], f32)
    cc2_out = nc.dram_tensor("cc2_out", [2], f32)
    GROUPS = [[0, 1, 2, 3], [4, 5, 6, 7]]

    with tile.TileContext(nc) as tc:
        with (
            tc.tile_pool(name="const", bufs=1) as cpool,
            tc.tile_pool(name="hpool", bufs=1) as hpool,
            tc.tile_pool(name="stat", bufs=1) as spool,
            tc.tile_pool(name="vecp", bufs=1) as vpool,
            tc.tile_pool(name="xin", bufs=2) as xpool,
            tc.tile_pool(name="yt", bufs=2) as ypool,
            tc.tile_pool(name="tmp", bufs=3) as tpool,
            tc.tile_pool(name="ybf", bufs=2) as ybpool,
            tc.tile_pool(name="yin", bufs=2) as yipool,
            tc.tile_pool(name="outp", bufs=3) as opool,
            tc.tile_pool(name="ps", bufs=6, space="PSUM") as pspool,
            tc.tile_pool(name="psr", bufs=1, space="PSUM") as psrpool,
        ):
            # ---------- phase 0: constants ----------
            w1sb = [cpool.tile([128, DIM], bf16, tag=f"w1_{j}", name=f"w1_{j}") for j in range(3)]
            w2lsb = [cpool.tile([128, DIM], bf16, tag=f"w2l_{j}", name=f"w2l_{j}") for j in range(3)]
            w2tsb = [cpool.tile([128, DIM], bf16, tag=f"w2t_{j}", name=f"w2t_{j}") for j in range(3)]
            w2hsb = [cpool.tile([128, DIM], bf16, tag=f"w2h_{j}", name=f"w2h_{j}") for j in range(3)]
            w3sb = [cpool.tile([128, DIM], bf16, tag=f"w3_{j}", name=f"w3_{j}") for j in range(3)]
            for j in range(3):
                sl = slice(j * 128, (j + 1) * 128)
                nc.sync.dma_start(out=w1sb[j][:], in_=w1t[sl, :])
                nc.sync.dma_start(out=w2lsb[j][:], in_=w2lt[sl, :])
                nc.sync.dma_start(out=w2tsb[j][:], in_=w2tt[sl, :])
                nc.sync.dma_start(out=w2hsb[j][:], in_=w2ht[sl, :])
                nc.sync.dma_start(out=w3sb[j][:], in_=w3t[sl, :])

            vt = cpool.tile([128, 8, 3], f32, tag="vecs", name="vecs")
            nc.gpsimd.dma_start(
                out=vt[:],
                in_=bass.AP(tensor=vecs.ap().tensor, offset=0,
                            ap=[[1, 128], [DIM, 8], [128, 3]]),
            )

            def vec(r, m):
                return vt[:, r, m:m + 1]

            hmb = cpool.tile([128, 2], f32, tag="hm", name="hm")
            nc.gpsimd.dma_start(
                out=hmb[:],
                in_=bass.AP(tensor=hm.ap().tensor, offset=0, ap=[[0, 128], [1, 2]]),
            )
            eps_t = cpool.tile([128, 1], f32, tag="eps", name="eps")
            nc.vector.memset(eps_t[:], EPS)
            ones = cpool.tile([128, 1], f32, tag="ones", name="ones")
            nc.vector.memset(ones[:], 1.0)

            hb = [hpool.tile([128, HBUF], bf16, tag=f"hb{m}", name=f"hb{m}") for m in range(3)]
            zsb = cpool.tile([128, 330], bf16, tag="zsb", name="zsb")
            nc.gpsimd.dma_start(
                out=zsb[:],
                in_=bass.AP(tensor=zpad.ap().tensor, offset=0,
                            ap=[[0, 128], [1, 330]]),
            )
            zv = zsb[:].rearrange("p (a b) -> p a b", a=DTOT)
            for m in range(3):
                nc.scalar.activation(out=hb[m][:, 0:1], in_=zsb[:, 0:1], func=AF.Copy)
                hv = hb[m][:, 1:].rearrange("p (d h w) -> p d h w", d=DTOT, h=33)
                nc.scalar.activation(out=hv[:, :, 32, :], in_=zv, func=AF.Copy)
                nc.scalar.activation(out=hv[:, :, :, 32], in_=zv, func=AF.Copy)

            st1 = [spool.tile([128, 16, 6], f32, tag=f"st1_{m}", name=f"st1_{m}") for m in range(3)]
            st2 = [spool.tile([128, 16, 6], f32, tag=f"st2_{m}", name=f"st2_{m}") for m in range(3)]

            def vtile(tag):
                return vpool.tile([128, 1], f32, tag=tag, name=tag)

            def vtile2(tag):
                return vpool.tile([128, 2], f32, tag=tag, name=tag)

            # ---------- phase 1: conv1 into padded h buffer (pre-norm) ----------
            for n in range(NT_IN):
                xt = [xpool.tile([128, 512], bf16, tag=f"xt{j}", name=f"xt{j}") for j in range(3)]
                for j in range(3):
                    nc.sync.dma_start(
                        out=xt[j][:],
                        in_=xs[j * 128:(j + 1) * 128, n * 512:(n + 1) * 512],
                    )
                d, half = n // 2, n % 2
                for m in range(3):
                    ps = pspool.tile([128, 512], f32, tag="ps", name="ps")
                    for j in range(3):
                        nc.tensor.matmul(
                            ps[:],
                            w1sb[j][:, m * 128:(m + 1) * 128],
                            xt[j][:],
                            start=(j == 0), stop=(j == 2),
                        )
                    off = 1 + d * SLICE + half * 16 * 33
                    dest = hb[m][:, off:off + 16 * 33].rearrange(
                        "p (h w) -> p h w", h=16)[:, :, 0:32]
                    nc.scalar.activation(
                        out=dest,
                        in_=ps[:].rearrange("p (h w) -> p h w", h=16),
                        func=AF.Copy,
                    )
                    if 2 <= n < 18:
                        nc.vector.bn_stats(out=st1[m][:, n - 2, :], in_=ps[:])

            # ---------- phase 1.5: gn1 stats + collective + scale/bias vecs ----
            sbq1 = [vtile2(f"sbq1_{m}") for m in range(3)]
            for m in range(3):
                mv = vtile2(f"mv1_{m}")
                nc.vector.bn_aggr(out=mv[:], in_=st1[m][:])
                # col0: sum with bias = 8192*(mean + b1)
                nc.vector.tensor_scalar(
                    out=sbq1[m][:, 0:1], in0=mv[:, 0:1],
                    scalar1=vec(VB1, m), scalar2=float(TOK_OUT),
                    op0=OP.add, op1=OP.mult,
                )
                # col1: sumsq with bias = 8192*var + sum^2/8192
                tsq = vtile(f"tsq1_{m}")
                nc.vector.tensor_mul(tsq[:], sbq1[m][:, 0:1], sbq1[m][:, 0:1])
                tv8 = vtile(f"tv81_{m}")
                nc.vector.tensor_scalar_mul(tv8[:], in0=mv[:, 1:2],
                                            scalar1=float(TOK_OUT))
                nc.vector.tensor_scalar(
                    out=sbq1[m][:, 1:2], in0=tsq[:],
                    scalar1=1.0 / TOK_OUT, scalar2=tv8[:],
                    op0=OP.mult, op1=OP.add,
                )
            psr = psrpool.tile([1, 2], f32, tag="psr1", name="psr1")
            for m in range(3):
                nc.tensor.matmul(psr[:], ones[:],
                                 sbq1[m][:],
                                 start=(m == 0), stop=(m == 2))
            prs = vpool.tile([1, 2], f32, tag="prs1", name="prs1")
            nc.vector.tensor_copy(out=prs[:], in_=psr[:])
            nc.sync.dma_start(out=cc1_in[:], in_=prs[:])
            nc.gpsimd.collective_compute(
                "AllReduce", OP.add, replica_groups=GROUPS,
                ins=[cc1_in.ap().opt()], outs=[cc1_out.ap().opt()],
            )
            gstat1 = vtile2("gstat1")
            nc.gpsimd.dma_start(
                out=gstat1[:],
                in_=bass.AP(tensor=cc1_out.ap().tensor, offset=0,
                            ap=[[0, 128], [1, 2]]),
            )
            mu1 = vtile("mu1")
            nc.vector.tensor_scalar_mul(mu1[:], in0=gstat1[:, 0:1], scalar1=1.0 / NTOT)
            m21 = vtile("m21")
            nc.vector.tensor_scalar_mul(m21[:], in0=gstat1[:, 1:2], scalar1=1.0 / NTOT)
            var1 = vtile("var1")
            nc.vector.tensor_mul(var1[:], mu1[:], mu1[:])
            nc.vector.tensor_sub(var1[:], m21[:], var1[:])
            sd1 = vtile("sd1")
            nc.scalar.activation(out=sd1[:], in_=var1[:], func=AF.Sqrt,
                                 bias=eps_t[:], scale=1.0)
            rstd1 = vtile("rstd1")
            nc.vector.reciprocal(rstd1[:], sd1[:])
            sv, tv = [], []
            svlo, tvlo, svhi, tvhi = [], [], [], []
            for m in range(3):
                s_m = vtile(f"sv_{m}")
                nc.vector.tensor_mul(s_m[:], vec(VG1, m), rstd1[:])
                t_m = vtile(f"tv_{m}")
                nc.vector.tensor_sub(t_m[:], vec(VB1, m), mu1[:])
                nc.vector.tensor_mul(t_m[:], t_m[:], s_m[:])
                nc.vector.tensor_add(t_m[:], t_m[:], vec(VBT1, m))
                sv.append(s_m)
                tv.append(t_m)
                for lst, src, col, nm in (
                    (svlo, s_m, 0, "svlo"), (tvlo, t_m, 0, "tvlo"),
                    (svhi, s_m, 1, "svhi"), (tvhi, t_m, 1, "tvhi"),
                ):
                    q = vtile(f"{nm}_{m}")
                    nc.vector.tensor_mul(q[:], src[:], hmb[:, col:col + 1])
                    lst.append(q)

            # ---------- phases 2+3 interleaved: gelu(gn1) then shifted convs --
            conv2 = [(w2lsb, 33, VB21), (w2tsb, SLICE, VB22), (w2hsb, 1, VB23)]

            def phase3_dout(do):
                for half in range(2):
                    nidx = (do - 1) * 2 + half
                    base = 1 + do * SLICE + half * 16 * 33
                    yts = [None] * 3
                    for a, (wsb, stp, bvrow) in enumerate(conv2):
                        for m in range(3):
                            ps = pspool.tile([128, 512], f32, tag="ps", name="ps")
                            for j in range(3):
                                off = base - (j - 1) * stp
                                rhs = hb[j][:, off:off + 16 * 33].rearrange(
                                    "p (h w) -> p h w", h=16)[:, :, 0:32]
                                nc.tensor.matmul(
                                    ps[:],
                                    wsb[j][:, m * 128:(m + 1) * 128],
                                    rhs,
                                    start=(j == 0), stop=(j == 2),
                                )
                            if a == 0:
                                yt = ypool.tile([128, 512], f32, tag=f"yt{m}", name=f"yt{m}")
                                yts[m] = yt
                                nc.scalar.activation(out=yt[:], in_=ps[:],
                                                     func=GELU, bias=vec(bvrow, m))
                            elif a == 1:
                                tmp = tpool.tile([128, 512], f32, tag="tmp", name="tmp")
                                nc.scalar.activation(out=tmp[:], in_=ps[:],
                                                     func=GELU, bias=vec(bvrow, m))
                                nc.vector.tensor_add(yts[m][:], yts[m][:], tmp[:])
                            else:
                                tmp = tpool.tile([128, 512], f32, tag="tmp", name="tmp")
                                nc.scalar.activation(out=tmp[:], in_=ps[:],
                                                     func=GELU, bias=vec(bvrow, m))
                                yb = ybpool.tile([128, 512], bf16, tag=f"yb{m}", name=f"yb{m}")
                                nc.vector.tensor_add(yb[:], yts[m][:], tmp[:])
                                nc.vector.bn_stats(out=st2[m][:, nidx, :], in_=yb[:])
                                nc.sync.dma_start(
                                    out=y_d[m][:, nidx * 512:(nidx + 1) * 512],
                                    in_=yb[:],
                                )

            for d in range(DTOT):
                for m in range(3):
                    ap = hb[m][:, 1 + d * SLICE:1 + (d + 1) * SLICE].rearrange(
                        "p (h w) -> p h w", h=33)[:, 0:32, 0:32]
                    if d == 0:
                        s_m, t_m = svlo[m], tvlo[m]
                    elif d == DTOT - 1:
                        s_m, t_m = svhi[m], tvhi[m]
                    else:
                        s_m, t_m = sv[m], tv[m]
                    nc.scalar.activation(out=ap, in_=ap, func=GELU,
                                         bias=t_m[:], scale=s_m[:])
                if d >= 2:
                    phase3_dout(d - 1)

            if debug:
                for m in range(3):
                    nc.sync.dma_start(out=dbg_h[m][:], in_=hb[m][:])
                nc.sync.dma_start(out=dbg_s[:, 0:2], in_=gstat1[:])
                nc.sync.dma_start(out=dbg_s[:, 2:3], in_=mu1[:])
                nc.sync.dma_start(out=dbg_s[:, 3:4], in_=rstd1[:])
                for m in range(3):
                    nc.sync.dma_start(out=dbg_s[:, 4 + m:5 + m], in_=sv[m][:])
                    nc.sync.dma_start(out=dbg_s[:, 7 + m:8 + m], in_=tv[m][:])

            # ---------- phase 3.5: gn2 stats + collective + epilogue vecs ----
            sbq2 = [vtile2(f"sbq2_{m}") for m in range(3)]
            for m in range(3):
                mv = vtile2(f"mv2_{m}")
                nc.vector.bn_aggr(out=mv[:], in_=st2[m][:])
                nc.vector.tensor_scalar_mul(sbq2[m][:, 0:1], in0=mv[:, 0:1],
                                            scalar1=float(TOK_OUT))
                tsq = vtile(f"tsq2_{m}")
                nc.vector.tensor_mul(tsq[:], mv[:, 0:1], mv[:, 0:1])
                nc.vector.tensor_add(tsq[:], tsq[:], mv[:, 1:2])
                nc.vector.tensor_scalar_mul(sbq2[m][:, 1:2], in0=tsq[:],
                                            scalar1=float(TOK_OUT))
            psr2 = psrpool.tile([1, 2], f32, tag="psr2", name="psr2")
            for m in range(3):
                nc.tensor.matmul(psr2[:], ones[:],
                                 sbq2[m][:],
                                 start=(m == 0), stop=(m == 2))
            prs2 = vpool.tile([1, 2], f32, tag="prs2", name="prs2")
            nc.vector.tensor_copy(out=prs2[:], in_=psr2[:])
            nc.sync.dma_start(out=cc2_in[:], in_=prs2[:])
            nc.gpsimd.collective_compute(
                "AllReduce", OP.add, replica_groups=GROUPS,
                ins=[cc2_in.ap().opt()], outs=[cc2_out.ap().opt()],
            )
            gstat2 = vtile2("gstat2")
            nc.gpsimd.dma_start(
                out=gstat2[:],
                in_=bass.AP(tensor=cc2_out.ap().tensor, offset=0,
                            ap=[[0, 128], [1, 2]]),
            )
            mu2 = vtile("mu2")
            nc.vector.tensor_scalar_mul(mu2[:], in0=gstat2[:, 0:1], scalar1=1.0 / NTOT)
            m22 = vtile("m22")
            nc.vector.tensor_scalar_mul(m22[:], in0=gstat2[:, 1:2], scalar1=1.0 / NTOT)
            var2 = vtile("var2")
            nc.vector.tensor_mul(var2[:], mu2[:], mu2[:])
            nc.vector.tensor_sub(var2[:], m22[:], var2[:])
            sd2 = vtile("sd2")
            nc.scalar.activation(out=sd2[:], in_=var2[:], func=AF.Sqrt,
                                 bias=eps_t[:], scale=1.0)
            rstd2 = vtile("rstd2")
            nc.vector.reciprocal(rstd2[:], sd2[:])
            p2 = vtile("p2")
            nc.vector.tensor_mul(p2[:], mu2[:], rstd2[:])
            cst = []
            for m in range(3):
                c_m = vtile(f"cst_{m}")
                nc.vector.tensor_mul(c_m[:], vec(VAV, m), p2[:])
                nc.vector.tensor_sub(c_m[:], vec(VBV, m), c_m[:])
                cst.append(c_m)

            # ---------- phase 4: final conv (bf16) + per-channel epilogue ----
            for n in range(NT_OUT):
                yin = [yipool.tile([128, 512], bf16, tag=f"yi{j}", name=f"yi{j}") for j in range(3)]
                for j in range(3):
                    nc.sync.dma_start(out=yin[j][:],
                                      in_=y_d[j][:, n * 512:(n + 1) * 512])
                for m in range(3):
                    ps = pspool.tile([128, 512], f32, tag="ps", name="ps")
                    for j in range(3):
                        nc.tensor.matmul(
                            ps[:],
                            w3sb[j][:, m * 128:(m + 1) * 128],
                            yin[j][:],
                            start=(j == 0), stop=(j == 2),
                        )
                    ot = opool.tile([128, 512], f32, tag="ot", name="ot")
                    nc.vector.tensor_scalar(
                        out=ot[:], in0=ps[:], scalar1=rstd2[:], scalar2=cst[m][:],
                        op0=OP.mult, op1=OP.add,
                    )
                    nc.sync.dma_start(
                        out=out_d[m * 128:(m + 1) * 128, n * 512:(n + 1) * 512],
                        in_=ot[:],
                    )

            if debug:
                for m in range(3):
                    nc.sync.dma_start(out=dbg_y[m][:], in_=y_d[m][:])

    nc.compile()
    return nc


def _prepare_in_maps(inputs):
    import ml_dtypes

    f = np.float32
    x = np.asarray(inputs["x"], f)
    w1 = np.asarray(inputs["w1"], f)
    b1 = np.asarray(inputs["b1"], f)
    g1 = np.asarray(inputs["g1"], f)
    bt1 = np.asarray(inputs["bt1"], f)
    w21 = np.asarray(inputs["w21"], f)
    b21 = np.asarray(inputs["b21"], f)
    w22 = np.asarray(inputs["w22"], f)
    b22 = np.asarray(inputs["b22"], f)
    w23 = np.asarray(inputs["w23"], f)
    b23 = np.asarray(inputs["b23"], f)
    g2 = np.asarray(inputs["g2"], f)
    bt2 = np.asarray(inputs["bt2"], f)
    w3 = np.asarray(inputs["w3"], f)
    b3 = np.asarray(inputs["b3"], f)

    w1t = np.ascontiguousarray(w1.T).astype(ml_dtypes.bfloat16)
    # x_lr shifts along H and uses w21; x_td along D uses w22; x_hd along W, w23
    w2lt = np.ascontiguousarray(w21.T).astype(ml_dtypes.bfloat16)
    w2tt = np.ascontiguousarray(w22.T).astype(ml_dtypes.bfloat16)
    w2ht = np.ascontiguousarray(w23.T).astype(ml_dtypes.bfloat16)
    w3g = w3 * g2[None, :]
    w3t = np.ascontiguousarray(w3g.T).astype(ml_dtypes.bfloat16)
    avec = w3 @ g2
    bvec = b3 + w3 @ bt2
    vecs = np.ascontiguousarray(
        np.stack([b1, g1, bt1, b21, b22, b23, avec, bvec]).astype(f))

    in_maps = []
    for core in range(8):
        b, d0 = core // 4, (core % 4) * DSH
        xsh = np.zeros((DIM, DTOT, R, R), f)
        lo, hi = d0 - 1, d0 + DSH + 1
        s0, s1 = max(lo, 0), min(hi, R)
        xsh[:, s0 - lo:s0 - lo + (s1 - s0)] = x[b, :, s0:s1]
        hmv = np.array([0.0 if d0 == 0 else 1.0,
                        0.0 if d0 + DSH == R else 1.0], f)
        in_maps.append(dict(
            xs=np.ascontiguousarray(xsh.reshape(DIM, TOK_IN)).astype(
                ml_dtypes.bfloat16),
            zpad=np.zeros(330, ml_dtypes.bfloat16),
            w1t=w1t, w2lt=w2lt, w2tt=w2tt, w2ht=w2ht, w3t=w3t,
            vecs=vecs, hm=hmv,
        ))
    return in_maps


def _gather(results):
    out = np.empty((B, DIM, R, R, R), np.float32)
    for core in range(8):
        b, d0 = core // 4, (core % 4) * DSH
        out[b, :, d0:d0 + DSH] = results[core]["out"].reshape(DIM, DSH, R, R)
    return out


def _run(inputs, trace=False, tmpdir=None):
    global _compiled
    if _compiled is None:
        _compiled = _build()
    from concourse import bass_utils

    in_maps = _prepare_in_maps(inputs)
    res = bass_utils.run_bass_kernel_spmd(
        _compiled, in_maps, core_ids=list(range(8)), trace=trace, tmpdir=tmpdir)
    return _gather(res.results), res


def kernel(**inputs) -> np.ndarray:
    out, _ = _run(inputs)
    return out

